# revision 15
# baseline (speedup 1.0000x reference)
"""Trainium2 Bass kernel for the dense U-Net dynamic-filter network.

Sharding: 8 cores = batch(4) x H-halves(2). Every layer keeps activations in
per-core DRAM "homes" shaped [nci, ci, n+2, W+2] (1-row halo + 1-col zero pad
ring). 3x3 convs consume the ring; halo rows are refreshed after each layer by
a pair AllGather + per-core 0/1 mask multiply (masks arrive as inputs, so the
instruction stream is identical on every core). Convs run as 9-shift
accumulated matmuls on the tensor engine in float32r (full-rate, ~1e-4
matmul error). conv1_1 uses a host-built im2col input (K=27). avgpool/up2 are
DVE/ACT stencils; up2 computes its own halo rows and applies edge-clamp
corrections via mask inputs. The final per-pixel 5x5 dynamic conv runs in a
y-on-partitions layout. Host side packs weights into matmul-ready layouts.
"""
import sys
sys.path.insert(0, '/opt/trn_rl_repo')
import numpy as np

N_CORES = 8

# name, Cin, Cout, n, W, relu, srcs[(home, nci)], ksize, nbands
CFG = [
    ("c11", 27, 64, 128, 256, True, [("x_i2c", 1)], 1, 8),
    ("c12", 64, 64, 128, 256, True, [("h_c11", 1)], 3, 8),
    ("c13", 64, 64, 128, 256, True, [("h_c12", 1)], 3, 8),
    ("c21", 64, 128, 64, 128, True, [("h_p1", 1)], 3, 2),
    ("c22", 128, 128, 64, 128, True, [("h_c21", 1)], 3, 2),
    ("c23", 128, 128, 64, 128, True, [("h_c22", 1)], 3, 2),
    ("c31", 128, 256, 32, 64, True, [("h_p2", 1)], 3, 1),
    ("c32", 256, 256, 32, 64, True, [("h_c31", 2)], 3, 1),
    ("c33", 256, 256, 32, 64, True, [("h_c32", 2)], 3, 1),
    ("c41", 256, 512, 16, 32, True, [("h_p3", 2)], 3, 1),
    ("c42", 512, 512, 16, 32, True, [("h_c41", 4)], 3, 1),
    ("c43", 512, 512, 16, 32, True, [("h_c42", 4)], 3, 1),
    ("c51", 512, 512, 8, 16, True, [("h_p4", 4)], 3, 1),
    ("c52", 512, 512, 8, 16, True, [("h_c51", 4)], 3, 1),
    ("c53", 512, 512, 8, 16, True, [("h_c52", 4)], 3, 1),
    ("c61", 1024, 512, 16, 32, True, [("h_c43", 4), ("h_u5", 4)], 3, 1),
    ("c62", 512, 512, 16, 32, True, [("h_c61", 4)], 3, 1),
    ("c63", 512, 512, 16, 32, True, [("h_c62", 4)], 3, 1),
    ("c71", 768, 256, 32, 64, True, [("h_c33", 2), ("h_u6", 4)], 3, 2),
    ("c72", 256, 256, 32, 64, True, [("h_c71", 2)], 3, 1),
    ("c73", 256, 256, 32, 64, True, [("h_c72", 2)], 3, 1),
    ("c81", 384, 75, 64, 128, True, [("h_c23", 1), ("h_u7", 2)], 3, 4),
    ("c82", 75, 75, 64, 128, True, [("h_c81", 1)], 3, 2),
    ("c83", 75, 75, 64, 128, True, [("h_c82", 1)], 3, 2),
    ("outc", 75, 75, 64, 128, False, [("h_c83", 1)], 1, 2),
]
CFGD = {c[0]: c for c in CFG}
CONV_PARAM_MAP = [
    ("c11", "conv1", 0), ("c12", "conv1", 1), ("c13", "conv1", 2),
    ("c21", "conv2", 0), ("c22", "conv2", 1), ("c23", "conv2", 2),
    ("c31", "conv3", 0), ("c32", "conv3", 1), ("c33", "conv3", 2),
    ("c41", "conv4", 0), ("c42", "conv4", 1), ("c43", "conv4", 2),
    ("c51", "conv5", 0), ("c52", "conv5", 1), ("c53", "conv5", 2),
    ("c61", "conv6", 0), ("c62", "conv6", 1), ("c63", "conv6", 2),
    ("c71", "conv7", 0), ("c72", "conv7", 1), ("c73", "conv7", 2),
    ("c81", "conv8", 0), ("c82", "conv8", 1), ("c83", "conv8", 2),
]
RCHUNK = {256: 2, 128: 4, 64: 8, 32: 16, 16: 8}      # for ksize==1 convs
RCH3 = {256: 1, 128: 3, 64: 7, 32: 8, 16: 8}         # 3x3: rows per chunk, rr*(W+2) <= 512

# stage sequence after each conv/pool: (kind, args)
#   conv: (name,) ; ag: (home, Ctot, n, W) ; pool: (src, dst, Ctot, n, W, nb)
#   up: (src, dst, Ctot, n, W, tb)
STAGES = [
    ("conv", "c11"), ("ag", "h_c11", 64, 128, 256),
    ("conv", "c12"), ("ag", "h_c12", 64, 128, 256),
    ("conv", "c13"), ("ag", "h_p1", 64, 64, 128),
    ("conv", "c21"), ("ag", "h_c21", 128, 64, 128),
    ("conv", "c22"), ("ag", "h_c22", 128, 64, 128),
    ("conv", "c23"), ("ag", "h_c23", 128, 64, 128), ("ag", "h_p2", 128, 32, 64),
    ("conv", "c31"), ("ag", "h_c31", 256, 32, 64),
    ("conv", "c32"), ("ag", "h_c32", 256, 32, 64),
    ("conv", "c33"), ("ag", "h_c33", 256, 32, 64), ("ag", "h_p3", 256, 16, 32),
    ("conv", "c41"), ("ag", "h_c41", 512, 16, 32),
    ("conv", "c42"), ("ag", "h_c42", 512, 16, 32),
    ("conv", "c43"), ("ag", "h_c43", 512, 16, 32), ("ag", "h_p4", 512, 8, 16),
    ("conv", "c51"), ("ag", "h_c51", 512, 8, 16),
    ("conv", "c52"), ("ag", "h_c52", 512, 8, 16),
    ("conv", "c53"), ("ag", "h_c53", 512, 8, 16),
    ("up", "h_c53", "h_u5", 512, 8, 16, 18),
    ("conv", "c61"), ("ag", "h_c61", 512, 16, 32),
    ("conv", "c62"), ("ag", "h_c62", 512, 16, 32),
    ("conv", "c63"), ("ag", "h_c63", 512, 16, 32),
    ("up", "h_c63", "h_u6", 512, 16, 32, 34),
    ("conv", "c71"), ("ag", "h_c71", 256, 32, 64),
    ("conv", "c72"), ("ag", "h_c72", 256, 32, 64),
    ("conv", "c73"), ("ag", "h_c73", 256, 32, 64),
    ("up", "h_c73", "h_u7", 256, 32, 64, 66),
    ("conv", "c81"), ("ag", "h_c81", 75, 64, 128),
    ("conv", "c82"), ("ag", "h_c82", 75, 64, 128),
    ("conv", "c83"), ("ag", "h_c83", 75, 64, 128),
    ("conv", "outc"),
    ("upt", "h_oc8", None, 75, 64, 128, 32),
    ("kc",),
]

# homes: key -> (Ctot, n, W)
HOMES = {
    "h_c11": (64, 128, 256), "h_c12": (64, 128, 256),
    "h_p1": (64, 64, 128),
    "h_c21": (128, 64, 128), "h_c22": (128, 64, 128), "h_c23": (128, 64, 128),
    "h_p2": (128, 32, 64),
    "h_c31": (256, 32, 64), "h_c32": (256, 32, 64), "h_c33": (256, 32, 64),
    "h_p3": (256, 16, 32),
    "h_c41": (512, 16, 32), "h_c42": (512, 16, 32), "h_c43": (512, 16, 32),
    "h_p4": (512, 8, 16),
    "h_c51": (512, 8, 16), "h_c52": (512, 8, 16), "h_c53": (512, 8, 16),
    "h_u5": (512, 16, 32),
    "h_c61": (512, 16, 32), "h_c62": (512, 16, 32), "h_c63": (512, 16, 32),
    "h_u6": (512, 32, 64),
    "h_c71": (256, 32, 64), "h_c72": (256, 32, 64), "h_c73": (256, 32, 64),
    "h_u7": (256, 64, 128),
    "h_c81": (75, 64, 128), "h_c82": (75, 64, 128), "h_c83": (75, 64, 128),
    "h_oc8": (75, 64, 128),
}

# per-conv extras: fused pools, full-row (halo-inclusive) outputs
CONV_EXTRAS = {
    "c13": {"pool_dst": "h_p1", "skip_home": True},
    "c23": {"pool_dst": "h_p2"},
    "c33": {"pool_dst": "h_p3"},
    "c43": {"pool_dst": "h_p4"},
    "outc": {"full_rows": True},
}


def nsplit(C):
    return (1, C) if C <= 128 else (C // 128, 128)


def pack_weights(params):
    out = {}
    for cname, pkey, li in CONV_PARAM_MAP:
        Wt = np.asarray(params[pkey][li][0], np.float32)
        bt = np.asarray(params[pkey][li][1], np.float32)
        Cout, Cin = Wt.shape[0], Wt.shape[1]
        if cname == "c11":
            w = Wt.transpose(2, 3, 1, 0).reshape(27, 64)[None, :, None, None, :]
            out["w_c11"] = np.ascontiguousarray(w, np.float32)
            out["b_c11"] = np.ascontiguousarray(bt.reshape(64, 1), np.float32)
            continue
        nci, ci = nsplit(Cin)
        nco, co = nsplit(Cout)
        w = Wt.reshape(nco, co, nci, ci, 9).transpose(0, 3, 2, 4, 1)
        out[f"w_{cname}"] = np.ascontiguousarray(w, np.float32)
        out[f"b_{cname}"] = np.ascontiguousarray(bt.reshape(nco, co).T, np.float32)
    wo = np.asarray(params["outc"][0], np.float32)[:, :, 0, 0]
    bo = np.asarray(params["outc"][1], np.float32)
    out["w_outc"] = np.ascontiguousarray(wo.T[None, :, None, None, :], np.float32)
    out["b_outc"] = np.ascontiguousarray(bo.reshape(75, 1), np.float32)
    return out


def per_core_inputs(data):
    data = np.asarray(data, np.float32)
    Bn = data.shape[0]
    cores = []
    for c in range(2 * Bn):
        b, s = c // 2, c % 2
        gy0 = s * 128
        p1 = np.pad(data[b], ((0, 0), (1, 1), (1, 1)))
        i2c = np.stack([p1[:, gy0 + dy:gy0 + dy + 128, dx:dx + 256]
                        for dy in range(3) for dx in range(3)], axis=0).reshape(27, 128, 256)
        p2 = np.pad(data[b], ((0, 0), (2, 2), (2, 2)))
        kc = np.ascontiguousarray(p2[:, gy0:gy0 + 132, :].transpose(1, 0, 2))
        m_top = 1.0 if s == 1 else 0.0
        m_bot = 1.0 if s == 0 else 0.0
        masks = np.zeros((128, 6), np.float32)
        masks[:, 0] = m_top
        masks[:, 1] = m_bot
        masks[:, 2] = 0.25 * (1 - m_top)
        masks[:, 3] = 0.25 * (1 - m_bot)
        masks[:, 4] = 1 - m_top
        masks[:, 5] = 1 - m_bot
        cores.append({"x_i2c": np.ascontiguousarray(i2c), "data_kc": kc, "masks": masks})
    return cores


def build(n_cores=N_CORES, with_cc=True, taps=(), stages=None):
    import concourse.bass as bass  # noqa: F401
    import concourse.tile as tile
    from concourse import bacc, mybir

    f32 = mybir.dt.float32
    f32r = mybir.dt.float32r
    ADD = mybir.AluOpType.add
    MULT = mybir.AluOpType.mult
    AF = mybir.ActivationFunctionType
    pairs = [[2 * i, 2 * i + 1] for i in range(n_cores // 2)]

    nc = bacc.Bacc("TRN2", target_bir_lowering=False, debug=False,
                   num_devices=n_cores)

    x_i2c = nc.dram_tensor("x_i2c", [27, 128, 256], f32, kind="ExternalInput")
    data_kc = nc.dram_tensor("data_kc", [132, 3, 260], f32, kind="ExternalInput")
    masks = nc.dram_tensor("masks", [128, 6], f32, kind="ExternalInput")
    w_in, b_in = {}, {}
    for name, Cin, Cout, n, W, relu, srcs, ksize, nb in CFG:
        nci_tot = sum(x[1] for x in srcs)
        ci = Cin // nci_tot
        nco, co = nsplit(Cout)
        nsh = 1 if ksize == 1 else 9
        w_in[name] = nc.dram_tensor(f"w_{name}", [nco, ci, nci_tot, nsh, co], f32,
                                    kind="ExternalInput")
        b_in[name] = nc.dram_tensor(f"b_{name}", [co, nco], f32, kind="ExternalInput")
    out_t = nc.dram_tensor("out", [3, 128, 256], f32, kind="ExternalOutput")
    tap_t = {k: nc.dram_tensor(f"tap_{k}", [nsplit(HOMES[k][0])[0], nsplit(HOMES[k][0])[1],
                                            HOMES[k][1] + 2, HOMES[k][2]], f32,
                               kind="ExternalOutput")
             for k in taps}

    with tile.TileContext(nc) as tc:
        from contextlib import ExitStack
        with ExitStack() as ctx:
            gp = ctx.enter_context(tc.tile_pool(name="glob", bufs=1))
            pspool = ctx.enter_context(tc.tile_pool(name="psum", bufs=8, space="PSUM"))
            agp = ctx.enter_context(tc.tile_pool(name="agst", bufs=2))
            drp = ctx.enter_context(tc.tile_pool(name="drbounce", bufs=2, space="DRAM"))
            drh = ctx.enter_context(tc.tile_pool(name="drhomes", bufs=1, space="DRAM"))

            m_sb = gp.tile([128, 6], f32, tag="m_sb")
            nc.sync.dma_start(m_sb[:], masks[:])

            homes = {"x_i2c": x_i2c}
            for key, (Ctot, n, W) in HOMES.items():
                nci, ci = nsplit(Ctot)
                homes[key] = drh.tile([nci, ci, n + 2, W], f32, tag=key, name=key)
            u8t = drh.tile([128, 75, 256], f32, tag="h_u8t", name="h_u8t")

            def emit_conv(name):
                _, Cin, Cout, n, W, relu, srcs, ksize, nb = CFGD[name]
                ex = CONV_EXTRAS.get(name, {})
                full_rows = ex.get("full_rows", False)
                pool_dst = homes[ex["pool_dst"]] if "pool_dst" in ex else None
                skip_home = ex.get("skip_home", False)
                nci_tot = sum(x[1] for x in srcs)
                ci = Cin // nci_tot
                nco, co = nsplit(Cout)
                nsh = 1 if ksize == 1 else 9
                rch = min(RCHUNK[W] if nsh == 1 else RCH3[W], n)
                br = n // nb
                raw = (srcs[0][0] == "x_i2c")
                dst = homes["h_oc8"] if name == "outc" else (
                    None if skip_home else homes[f"h_{name}"])
                orows = br + 2 if full_rows else br
                with tc.tile_pool(name=f"p_{name}", bufs=2) as lp, \
                     tc.tile_pool(name=f"pw_{name}", bufs=2) as wp:
                    b_t = lp.tile([co, nco], f32, tag="b")
                    nc.sync.dma_start(b_t[:], b_in[name][:])
                    for ib in range(nb):
                        rb = ib * br
                        wp2 = W + 2
                        if raw:
                            x_t = lp.tile([ci, br, W], f32r, tag="x")
                            nc.sync.dma_start(x_t[:], x_i2c[:, rb:rb + br, :].bitcast(f32r))
                        elif nsh == 1:
                            x_t = lp.tile([ci, nci_tot, br + 2, W], f32r, tag="x")
                            gg = 0
                            for hname, hnci in srcs:
                                hm = homes[hname]
                                nc.sync.dma_start(
                                    x_t[:, gg:gg + hnci],
                                    hm[:, :, rb:rb + br + 2, :]
                                    .rearrange("g c r w -> c g r w").bitcast(f32r))
                                gg += hnci
                        else:
                            # flat im2col rows of width W+2; the 2 extra cols are
                            # zeroed so every 3x3 shift is one contiguous matmul
                            # with exact zero-pad column semantics. Row 0 / row
                            # br+3 are junk pad rows (only their zero cols and a
                            # single wrap element reach the psum garbage cols).
                            x_t = lp.tile([ci, nci_tot, br + 4, wp2], f32r, tag="x")
                            nc.vector.memset(x_t[:, :, :, W:W + 2].bitcast(f32), 0.0)
                            gg = 0
                            for hname, hnci in srcs:
                                hm = homes[hname]
                                for g in range(hnci):
                                    nc.sync.dma_start(
                                        x_t[:, gg, 1:br + 3, 0:W],
                                        hm[g, :, rb:rb + br + 2, :].bitcast(f32r))
                                    gg += 1
                        chunks = [(rc, min(rch, orows - rc))
                                  for rc in range(0, orows, rch)]
                        # multi-group convs loop groups outer (one small weight
                        # tile live at a time); psum tiles persist across groups
                        gg_outer = (nci_tot >= 2)
                        assert not gg_outer or len(chunks) <= 8
                        for o in range(nco):
                            o_t = lp.tile([co, orows, W], f32, tag="o")
                            if gg_outer:
                                pss = []
                                for k, (rc, rr) in enumerate(chunks):
                                    pss.append(pspool.tile([128, rch, wp2], f32,
                                                           tag="ps", name=f"ps{k}"))
                                for gg in range(nci_tot):
                                    w_t = wp.tile([ci, nsh, co], f32r, tag="w")
                                    nc.sync.dma_start(
                                        w_t[:], w_in[name][o, :, gg].bitcast(f32r))
                                    x_flat = x_t[:, gg].rearrange("c r w -> c (r w)")
                                    for k, (rc, rr) in enumerate(chunks):
                                        for s in range(9):
                                            dy, dx = s // 3, s % 3
                                            a = (1 + rc + dy) * wp2 + dx - 1
                                            rhs = x_flat[:, a:a + rr * wp2]
                                            nc.tensor.matmul(
                                                pss[k][:co, :rr], w_t[:, s, :], rhs,
                                                start=(gg == 0 and s == 0),
                                                stop=(gg == nci_tot - 1 and s == 8))
                                for k, (rc, rr) in enumerate(chunks):
                                    nc.scalar.activation(o_t[:, rc:rc + rr, :],
                                                         pss[k][:co, :rr, 0:W],
                                                         AF.Relu, bias=b_t[:, o:o + 1])
                            else:
                                w_t = wp.tile([ci, nsh, co], f32r, tag="w")
                                nc.sync.dma_start(w_t[:],
                                                  w_in[name][o, :, 0].bitcast(f32r))
                                for rc, rr in chunks:
                                    if nsh == 1:
                                        ps = pspool.tile([128, rch, W], f32, tag="ps")
                                        pvalid = ps[:co, :rr]
                                        if raw:
                                            rhs = x_t[:, rc:rc + rr, :]
                                        elif full_rows:
                                            rhs = x_t[:, 0, rc:rc + rr, :]
                                        else:
                                            rhs = x_t[:, 0, rc + 1:rc + 1 + rr, :]
                                        nc.tensor.matmul(pvalid, w_t[:, 0, :], rhs,
                                                         start=True, stop=True)
                                    else:
                                        ps = pspool.tile([128, rch, wp2], f32, tag="ps")
                                        pvalid = ps[:co, :rr, 0:W]
                                        x_flat = x_t[:, 0].rearrange("c r w -> c (r w)")
                                        for s in range(9):
                                            dy, dx = s // 3, s % 3
                                            a = (1 + rc + dy) * wp2 + dx - 1
                                            rhs = x_flat[:, a:a + rr * wp2]
                                            nc.tensor.matmul(ps[:co, :rr], w_t[:, s, :],
                                                             rhs, start=(s == 0),
                                                             stop=(s == 8))
                                    if relu:
                                        nc.scalar.activation(o_t[:, rc:rc + rr, :], pvalid,
                                                             AF.Relu, bias=b_t[:, o:o + 1])
                                    else:
                                        nc.vector.tensor_scalar_add(o_t[:, rc:rc + rr, :],
                                                                    pvalid, b_t[:, o:o + 1])
                            if full_rows:
                                nc.sync.dma_start(dst[o, :, rb:rb + br + 2, :], o_t[:])
                            elif dst is not None:
                                nc.sync.dma_start(dst[o, :, 1 + rb:1 + rb + br, :], o_t[:])
                            if pool_dst is not None:
                                pt1 = lp.tile([co, br // 2, W], f32, tag="pt1")
                                nc.vector.tensor_tensor(pt1[:], o_t[:, 0:br:2, :],
                                                        o_t[:, 1:br:2, :], ADD)
                                pt2 = lp.tile([co, br // 2, W // 2], f32, tag="pt2")
                                nc.vector.tensor_tensor(pt2[:], pt1[:, :, 0:W:2],
                                                        pt1[:, :, 1:W:2], ADD)
                                po = lp.tile([co, br // 2, W // 2], f32, tag="po")
                                nc.scalar.activation(po[:], pt2[:], AF.Copy, scale=0.25)
                                nc.sync.dma_start(
                                    pool_dst[o, :, 1 + rb // 2:1 + rb // 2 + br // 2, :],
                                    po[:])

            def emit_ag(key, Ctot, n, W):
                nci, ci = nsplit(Ctot)
                hm = homes[key]
                C = nci * ci
                bin_ = drp.tile([2, C, W], f32, tag="bin", name=f"bin_{key}")
                bout = drp.tile([2, 2, C, W], f32, tag="bout", name=f"bout_{key}")
                nc.sync.dma_start(bin_[0], hm[:, :, 1, :].rearrange("g c w -> (g c) w"))
                nc.sync.dma_start(bin_[1], hm[:, :, n, :].rearrange("g c w -> (g c) w"))
                if with_cc:
                    nc.gpsimd.collective_compute(
                        "AllGather", mybir.AluOpType.bypass, replica_groups=pairs,
                        ins=[bin_.opt()], outs=[bout.opt()])
                else:
                    nc.sync.dma_start(bout[0], bin_[:])
                    nc.sync.dma_start(bout[1], bin_[:])
                st = agp.tile([ci, 2, nci, W], f32, tag="st")
                nc.sync.dma_start(st[:, 0], bout[0, 1].rearrange("(g c) w -> c g w", c=ci))
                nc.sync.dma_start(st[:, 1], bout[1, 0].rearrange("(g c) w -> c g w", c=ci))
                nc.vector.tensor_scalar_mul(st[:, 0], st[:, 0], m_sb[:ci, 0:1])
                nc.vector.tensor_scalar_mul(st[:, 1], st[:, 1], m_sb[:ci, 1:2])
                nc.sync.dma_start(hm[:, :, 0, :].rearrange("g c w -> c g w"), st[:, 0])
                nc.sync.dma_start(hm[:, :, n + 1, :].rearrange("g c w -> c g w"), st[:, 1])

            def emit_pool(srck, dstk, Ctot, n, W, nb):
                nci, ci = nsplit(Ctot)
                hW = W // 2
                br = n // nb
                src, dst = homes[srck], homes[dstk]
                with tc.tile_pool(name=f"pl_{dstk}", bufs=2) as lp:
                    for g in range(nci):
                        for ib in range(nb):
                            rb = ib * br
                            x = lp.tile([ci, br, W], f32, tag="x")
                            nc.sync.dma_start(x[:], src[g, :, 1 + rb:1 + rb + br, :])
                            t1 = lp.tile([ci, br // 2, W], f32, tag="t1")
                            nc.vector.tensor_tensor(t1[:], x[:, 0:br:2, :], x[:, 1:br:2, :], ADD)
                            t2 = lp.tile([ci, br // 2, hW], f32, tag="t2")
                            nc.vector.tensor_tensor(t2[:], t1[:, :, 0:W:2],
                                                    t1[:, :, 1:W:2], ADD)
                            o = lp.tile([ci, br // 2, hW], f32, tag="o")
                            nc.scalar.activation(o[:], t2[:], AF.Copy, scale=0.25)
                            nc.sync.dma_start(
                                dst[g, :, 1 + rb // 2:1 + rb // 2 + br // 2, :], o[:])

            def emit_up2(srck, dstk, Ctot, n, W, tb, transposed=False):
                nci, ci = nsplit(Ctot)
                Wp2 = W
                n2 = 2 * n + 2
                src = homes[srck]
                dst = u8t if transposed else homes[dstk]
                with tc.tile_pool(name=f"up_{dstk}", bufs=1) as lp:
                    for g in range(nci):
                        for t0 in range(0, n2, tb):
                            t1_ = min(t0 + tb, n2)
                            tbn = t1_ - t0
                            k0 = t0 // 2
                            kn = t1_ // 2 - k0 + 1
                            half = tbn // 2
                            x = lp.tile([ci, tb // 2 + 1, Wp2], f32, tag="x")
                            nc.sync.dma_start(x[:, :kn], src[g, :, k0:k0 + kn, :])
                            Bq = lp.tile([ci, tb // 2 + 1, Wp2], f32, tag="B")
                            nc.vector.tensor_scalar_mul(Bq[:, :kn], x[:, :kn], 0.25)
                            nc.scalar.activation(x[:, :kn], x[:, :kn], AF.Copy, scale=0.75)
                            T = lp.tile([ci, tb, Wp2], f32, tag="T")
                            nc.vector.tensor_tensor(T[:, 0:tbn:2, :], x[:, 0:half, :],
                                                    Bq[:, 1:half + 1, :], ADD)
                            nc.vector.tensor_tensor(T[:, 1:tbn:2, :], x[:, 1:half + 1, :],
                                                    Bq[:, 0:half, :], ADD)
                            if t0 == 0 and not transposed:
                                nc.vector.tensor_scalar_mul(T[:, 0:1, :], T[:, 0:1, :],
                                                            m_sb[:ci, 0:1])
                            if t0 <= 1 < t1_:
                                c1t = lp.tile([ci, 1, Wp2], f32, tag="c1")
                                nc.vector.tensor_scalar_mul(c1t[:], Bq[:, 1 - k0:2 - k0, :],
                                                            m_sb[:ci, 4:5])
                                nc.vector.tensor_tensor(T[:, 1 - t0:2 - t0, :],
                                                        T[:, 1 - t0:2 - t0, :], c1t[:], ADD)
                            if t0 <= 2 * n < t1_:
                                c2t = lp.tile([ci, 1, Wp2], f32, tag="c2")
                                nc.vector.tensor_scalar_mul(c2t[:], Bq[:, n - k0:n - k0 + 1, :],
                                                            m_sb[:ci, 5:6])
                                lo = 2 * n - t0
                                nc.vector.tensor_tensor(T[:, lo:lo + 1, :],
                                                        T[:, lo:lo + 1, :], c2t[:], ADD)
                            if t1_ == n2 and not transposed:
                                nc.vector.tensor_scalar_mul(T[:, tbn - 1:tbn, :],
                                                            T[:, tbn - 1:tbn, :], m_sb[:ci, 1:2])
                            B2 = lp.tile([ci, tb, Wp2], f32, tag="B2")
                            nc.vector.tensor_scalar_mul(B2[:, :tbn], T[:, :tbn], 0.25)
                            nc.scalar.activation(T[:, :tbn], T[:, :tbn], AF.Copy, scale=0.75)
                            Y = lp.tile([ci, tb, 2 * W], f32, tag="Y")
                            nc.vector.tensor_tensor(Y[:, :tbn, 2:2 * W:2], T[:, :tbn, 1:W],
                                                    B2[:, :tbn, 0:W - 1], ADD)
                            nc.vector.tensor_tensor(Y[:, :tbn, 0:1], T[:, :tbn, 0:1],
                                                    B2[:, :tbn, 0:1], ADD)
                            nc.vector.tensor_tensor(Y[:, :tbn, 1:2 * W - 1:2], T[:, :tbn, 0:W - 1],
                                                    B2[:, :tbn, 1:W], ADD)
                            nc.vector.tensor_tensor(Y[:, :tbn, 2 * W - 1:2 * W],
                                                    T[:, :tbn, W - 1:W],
                                                    B2[:, :tbn, W - 1:W], ADD)
                            if transposed:
                                loc0 = 1 if t0 == 0 else 0
                                loc1 = tbn - 1 if t1_ == n2 else tbn
                                y0 = t0 + loc0 - 1
                                nc.sync.dma_start(
                                    dst[y0:y0 + (loc1 - loc0)]
                                    .rearrange("y c x -> c y x"),
                                    Y[:, loc0:loc1])
                            else:
                                nc.sync.dma_start(dst[g, :, t0:t1_, :], Y[:, :tbn])

            def emit_kc():
                with tc.tile_pool(name="kc", bufs=1) as lp:
                    ct = lp.tile([128, 75, 256], f32, tag="ct")
                    nc.sync.dma_start(ct[:], u8t[:])
                    acc = lp.tile([128, 3, 256], f32, tag="acc")
                    tmp = lp.tile([128, 3, 256], f32, tag="tmp")
                    first = True
                    for i in range(5):
                        d_i = lp.tile([128, 3, 260], f32, tag=f"d{i}")
                        nc.sync.dma_start(d_i[:], data_kc[i:i + 128])
                        for j in range(5):
                            kk = i * 5 + j
                            if first:
                                nc.vector.tensor_tensor(acc[:], ct[:, kk * 3:(kk + 1) * 3, :],
                                                        d_i[:, :, j:j + 256], MULT)
                                first = False
                            else:
                                nc.vector.tensor_tensor(tmp[:], ct[:, kk * 3:(kk + 1) * 3, :],
                                                        d_i[:, :, j:j + 256], MULT)
                                nc.vector.tensor_tensor(acc[:], acc[:], tmp[:], ADD)
                    nc.sync.dma_start(out_t.rearrange("c y x -> y c x"), acc[:])

            for st in (STAGES if stages is None else STAGES[:stages]):
                if st[0] == "conv":
                    emit_conv(st[1])
                elif st[0] == "ag":
                    emit_ag(*st[1:])
                elif st[0] == "pool":
                    emit_pool(*st[1:])
                elif st[0] == "up":
                    emit_up2(*st[1:])
                elif st[0] == "upt":
                    emit_up2(*st[1:], transposed=True)
                elif st[0] == "kc":
                    emit_kc()

            for k in taps:
                hm = homes[k]
                nci, ci = nsplit(HOMES[k][0])
                for g in range(nci):
                    nc.sync.dma_start(tap_t[k][g], hm[g])

    nc.finalize()
    return nc


class SpmdRunner:
    def __init__(self, nc, n_cores, sharded_names):
        import jax
        import numpy as _np
        from jax.sharding import Mesh, PartitionSpec
        from jax.experimental.shard_map import shard_map
        import concourse.mybir as mybir
        from concourse.bass2jax import (_bass_exec_p, partition_id_tensor,
                                        install_neuronx_cc_hook)
        install_neuronx_cc_hook()
        self.jax = jax
        self.n_cores = n_cores
        self.sharded = set(sharded_names)
        partition_name = nc.partition_id_tensor.name if nc.partition_id_tensor else None
        in_names, out_names, out_avals, zero_outs = [], [], [], []
        for alloc in nc.m.functions[0].allocations:
            if not isinstance(alloc, mybir.MemoryLocationSet):
                continue
            name = alloc.memorylocations[0].name
            if alloc.kind == "ExternalInput":
                if name != partition_name:
                    in_names.append(name)
            elif alloc.kind == "ExternalOutput":
                out_names.append(name)
                shape = tuple(alloc.tensor_shape)
                dtype = mybir.dt.np(alloc.dtype)
                out_avals.append(jax.core.ShapedArray(shape, dtype))
                zero_outs.append(_np.zeros(shape, dtype))
        self.in_names, self.out_names = in_names, out_names
        self.out_avals, self.zero_outs = out_avals, zero_outs
        n_params, n_outs = len(in_names), len(out_avals)
        all_in = list(in_names) + list(out_names)
        if partition_name is not None:
            all_in.append(partition_name)
        all_in = tuple(all_in)

        def _body(*args):
            operands = list(args)
            if partition_name is not None:
                operands.append(partition_id_tensor())
            outs = _bass_exec_p.bind(
                *operands, out_avals=tuple(out_avals), in_names=all_in,
                out_names=tuple(out_names), lowering_input_output_aliases=(),
                sim_require_finite=False, sim_require_nnan=False, nc=nc)
            return tuple(outs)

        devices = jax.devices()[:n_cores]
        assert len(devices) == n_cores
        mesh = Mesh(_np.asarray(devices), ("core",))
        in_specs = tuple(
            PartitionSpec("core") if nm in self.sharded else PartitionSpec()
            for nm in in_names) + (PartitionSpec("core"),) * n_outs
        out_specs = (PartitionSpec("core"),) * n_outs
        self._fn = jax.jit(
            shard_map(_body, mesh=mesh, in_specs=in_specs, out_specs=out_specs,
                      check_rep=False), keep_unused=True)

    def prep_args(self, in_maps):
        import numpy as _np
        n = self.n_cores
        args = []
        for nm in self.in_names:
            if nm in self.sharded:
                args.append(_np.concatenate(
                    [_np.asarray(in_maps[c][nm]) for c in range(n)], axis=0))
            else:
                args.append(_np.asarray(in_maps[0][nm]))
        args += [_np.zeros((n * z.shape[0], *z.shape[1:]), z.dtype)
                 for z in self.zero_outs]
        return args

    def run(self, in_maps):
        import numpy as _np
        n = self.n_cores
        outs = self._fn(*self.prep_args(in_maps))
        self.jax.block_until_ready(outs)
        return [
            {k: _np.asarray(outs[i]).reshape(n, *self.out_avals[i].shape)[c]
             for i, k in enumerate(self.out_names)}
            for c in range(n)
        ]

    def timeit(self, in_maps, reps=10):
        import time
        args = self.prep_args(in_maps)
        outs = self._fn(*args)
        self.jax.block_until_ready(outs)
        ts = []
        for _ in range(reps):
            t0 = time.perf_counter()
            outs = self._fn(*args)
            self.jax.block_until_ready(outs)
            ts.append(time.perf_counter() - t0)
        return ts


_STATE = {}


def _get_runner(taps=()):
    key = ("runner", tuple(taps))
    if key not in _STATE:
        nc = build(N_CORES, with_cc=True, taps=taps)
        _STATE[key] = SpmdRunner(nc, N_CORES,
                                 sharded_names=["x_i2c", "data_kc", "masks"])
    return _STATE[key]


def kernel(data, params):
    data = np.asarray(data, np.float32)
    Bn = data.shape[0]
    assert Bn * 2 == N_CORES
    wpk = pack_weights(params)
    cores = per_core_inputs(data)
    in_maps = []
    for c in range(N_CORES):
        m = dict(cores[c])
        m.update(wpk)
        in_maps.append(m)
    runner = _get_runner(taps=_STATE.get("taps", ()))
    res = runner.run(in_maps)
    full = np.zeros((Bn, 3, 256, 256), np.float32)
    for c in range(N_CORES):
        b, s = c // 2, c % 2
        full[b, :, s * 128:(s + 1) * 128, :] = res[c]["out"]
    _STATE["last_res"] = res
    return full


# revision 16
# speedup vs baseline: 227.3104x; 227.3104x over previous
"""Trainium2 Bass kernel for the dense U-Net dynamic-filter network.

Sharding: 8 cores = batch(4) x H-halves(2). Every layer keeps activations in
per-core DRAM "homes" shaped [nci, ci, n+2, W+2] (1-row halo + 1-col zero pad
ring). 3x3 convs consume the ring; halo rows are refreshed after each layer by
a pair AllGather + per-core 0/1 mask multiply (masks arrive as inputs, so the
instruction stream is identical on every core). Convs run as 9-shift
accumulated matmuls on the tensor engine in float32r (full-rate, ~1e-4
matmul error). conv1_1 uses a host-built im2col input (K=27). avgpool/up2 are
DVE/ACT stencils; up2 computes its own halo rows and applies edge-clamp
corrections via mask inputs. The final per-pixel 5x5 dynamic conv runs in a
y-on-partitions layout. Host side packs weights into matmul-ready layouts.
"""
import sys
sys.path.insert(0, '/opt/trn_rl_repo')
import numpy as np

N_CORES = 8

# name, Cin, Cout, n, W, relu, srcs[(home, nci)], ksize, nbands
CFG = [
    ("c11", 27, 64, 128, 256, True, [("x_i2c", 1)], 1, 8),
    ("c12", 64, 64, 128, 256, True, [("h_c11", 1)], 3, 8),
    ("c13", 64, 64, 128, 256, True, [("h_c12", 1)], 3, 8),
    ("c21", 64, 128, 64, 128, True, [("h_p1", 1)], 3, 2),
    ("c22", 128, 128, 64, 128, True, [("h_c21", 1)], 3, 2),
    ("c23", 128, 128, 64, 128, True, [("h_c22", 1)], 3, 2),
    ("c31", 128, 256, 32, 64, True, [("h_p2", 1)], 3, 1),
    ("c32", 256, 256, 32, 64, True, [("h_c31", 2)], 3, 1),
    ("c33", 256, 256, 32, 64, True, [("h_c32", 2)], 3, 1),
    ("c41", 256, 512, 16, 32, True, [("h_p3", 2)], 3, 1),
    ("c42", 512, 512, 16, 32, True, [("h_c41", 4)], 3, 1),
    ("c43", 512, 512, 16, 32, True, [("h_c42", 4)], 3, 1),
    ("c51", 512, 512, 8, 16, True, [("h_p4", 4)], 3, 1),
    ("c52", 512, 512, 8, 16, True, [("h_c51", 4)], 3, 1),
    ("c53", 512, 512, 8, 16, True, [("h_c52", 4)], 3, 1),
    ("c61", 1024, 512, 16, 32, True, [("h_c43", 4), ("h_u5", 4)], 3, 1),
    ("c62", 512, 512, 16, 32, True, [("h_c61", 4)], 3, 1),
    ("c63", 512, 512, 16, 32, True, [("h_c62", 4)], 3, 1),
    ("c71", 768, 256, 32, 64, True, [("h_c33", 2), ("h_u6", 4)], 3, 2),
    ("c72", 256, 256, 32, 64, True, [("h_c71", 2)], 3, 1),
    ("c73", 256, 256, 32, 64, True, [("h_c72", 2)], 3, 1),
    ("c81", 384, 75, 64, 128, True, [("h_c23", 1), ("h_u7", 2)], 3, 4),
    ("c82", 75, 75, 64, 128, True, [("h_c81", 1)], 3, 2),
    ("c83", 75, 75, 64, 128, True, [("h_c82", 1)], 3, 2),
    ("outc", 75, 75, 64, 128, False, [("h_c83", 1)], 1, 2),
]
CFGD = {c[0]: c for c in CFG}
CONV_PARAM_MAP = [
    ("c11", "conv1", 0), ("c12", "conv1", 1), ("c13", "conv1", 2),
    ("c21", "conv2", 0), ("c22", "conv2", 1), ("c23", "conv2", 2),
    ("c31", "conv3", 0), ("c32", "conv3", 1), ("c33", "conv3", 2),
    ("c41", "conv4", 0), ("c42", "conv4", 1), ("c43", "conv4", 2),
    ("c51", "conv5", 0), ("c52", "conv5", 1), ("c53", "conv5", 2),
    ("c61", "conv6", 0), ("c62", "conv6", 1), ("c63", "conv6", 2),
    ("c71", "conv7", 0), ("c72", "conv7", 1), ("c73", "conv7", 2),
    ("c81", "conv8", 0), ("c82", "conv8", 1), ("c83", "conv8", 2),
]
RCHUNK = {256: 2, 128: 4, 64: 8, 32: 16, 16: 8}      # for ksize==1 convs
RCH3 = {256: 1, 128: 3, 64: 7, 32: 8, 16: 8}         # 3x3: rows per chunk, rr*(W+2) <= 512

# stage sequence after each conv/pool: (kind, args)
#   conv: (name,) ; ag: (home, Ctot, n, W) ; pool: (src, dst, Ctot, n, W, nb)
#   up: (src, dst, Ctot, n, W, tb)
STAGES = [
    ("conv", "c11"), ("ag", "h_c11", 64, 128, 256),
    ("conv", "c12"), ("ag", "h_c12", 64, 128, 256),
    ("conv", "c13"), ("ag", "h_p1", 64, 64, 128),
    ("conv", "c21"), ("ag", "h_c21", 128, 64, 128),
    ("conv", "c22"), ("ag", "h_c22", 128, 64, 128),
    ("conv", "c23"), ("ag", "h_c23", 128, 64, 128), ("ag", "h_p2", 128, 32, 64),
    ("conv", "c31"), ("ag", "h_c31", 256, 32, 64),
    ("conv", "c32"), ("ag", "h_c32", 256, 32, 64),
    ("conv", "c33"), ("ag", "h_c33", 256, 32, 64), ("ag", "h_p3", 256, 16, 32),
    ("conv", "c41"), ("ag", "h_c41", 512, 16, 32),
    ("conv", "c42"), ("ag", "h_c42", 512, 16, 32),
    ("conv", "c43"), ("ag", "h_c43", 512, 16, 32), ("ag", "h_p4", 512, 8, 16),
    ("conv", "c51"), ("ag", "h_c51", 512, 8, 16),
    ("conv", "c52"), ("ag", "h_c52", 512, 8, 16),
    ("conv", "c53"), ("ag", "h_c53", 512, 8, 16),
    ("up", "h_c53", "h_u5", 512, 8, 16, 18),
    ("conv", "c61"), ("ag", "h_c61", 512, 16, 32),
    ("conv", "c62"), ("ag", "h_c62", 512, 16, 32),
    ("conv", "c63"), ("ag", "h_c63", 512, 16, 32),
    ("up", "h_c63", "h_u6", 512, 16, 32, 34),
    ("conv", "c71"), ("ag", "h_c71", 256, 32, 64),
    ("conv", "c72"), ("ag", "h_c72", 256, 32, 64),
    ("conv", "c73"), ("ag", "h_c73", 256, 32, 64),
    ("up", "h_c73", "h_u7", 256, 32, 64, 66),
    ("conv", "c81"), ("ag", "h_c81", 75, 64, 128),
    ("conv", "c82"), ("ag", "h_c82", 75, 64, 128),
    ("conv", "c83"), ("ag", "h_c83", 75, 64, 128),
    ("conv", "outc"),
    ("upt", "h_oc8", None, 75, 64, 128, 32),
    ("kc",),
]

# homes: key -> (Ctot, n, W)
HOMES = {
    "h_c11": (64, 128, 256), "h_c12": (64, 128, 256),
    "h_p1": (64, 64, 128),
    "h_c21": (128, 64, 128), "h_c22": (128, 64, 128), "h_c23": (128, 64, 128),
    "h_p2": (128, 32, 64),
    "h_c31": (256, 32, 64), "h_c32": (256, 32, 64), "h_c33": (256, 32, 64),
    "h_p3": (256, 16, 32),
    "h_c41": (512, 16, 32), "h_c42": (512, 16, 32), "h_c43": (512, 16, 32),
    "h_p4": (512, 8, 16),
    "h_c51": (512, 8, 16), "h_c52": (512, 8, 16), "h_c53": (512, 8, 16),
    "h_u5": (512, 16, 32),
    "h_c61": (512, 16, 32), "h_c62": (512, 16, 32), "h_c63": (512, 16, 32),
    "h_u6": (512, 32, 64),
    "h_c71": (256, 32, 64), "h_c72": (256, 32, 64), "h_c73": (256, 32, 64),
    "h_u7": (256, 64, 128),
    "h_c81": (75, 64, 128), "h_c82": (75, 64, 128), "h_c83": (75, 64, 128),
    "h_oc8": (75, 64, 128),
}

# per-conv extras: fused pools, full-row (halo-inclusive) outputs
CONV_EXTRAS = {
    "c13": {"pool_dst": "h_p1", "skip_home": True},
    "c23": {"pool_dst": "h_p2"},
    "c33": {"pool_dst": "h_p3"},
    "c43": {"pool_dst": "h_p4"},
    "outc": {"full_rows": True},
}


def nsplit(C):
    return (1, C) if C <= 128 else (C // 128, 128)


def pack_weights(params):
    out = {}
    for cname, pkey, li in CONV_PARAM_MAP:
        Wt = np.asarray(params[pkey][li][0], np.float32)
        bt = np.asarray(params[pkey][li][1], np.float32)
        Cout, Cin = Wt.shape[0], Wt.shape[1]
        if cname == "c11":
            w = Wt.transpose(2, 3, 1, 0).reshape(27, 64)[None, :, None, None, :]
            out["w_c11"] = np.ascontiguousarray(w, np.float32)
            out["b_c11"] = np.ascontiguousarray(bt.reshape(64, 1), np.float32)
            continue
        nci, ci = nsplit(Cin)
        nco, co = nsplit(Cout)
        w = Wt.reshape(nco, co, nci, ci, 9).transpose(0, 3, 2, 4, 1)
        out[f"w_{cname}"] = np.ascontiguousarray(w, np.float32)
        out[f"b_{cname}"] = np.ascontiguousarray(bt.reshape(nco, co).T, np.float32)
    wo = np.asarray(params["outc"][0], np.float32)[:, :, 0, 0]
    bo = np.asarray(params["outc"][1], np.float32)
    out["w_outc"] = np.ascontiguousarray(wo.T[None, :, None, None, :], np.float32)
    out["b_outc"] = np.ascontiguousarray(bo.reshape(75, 1), np.float32)
    return out


def per_core_inputs(data):
    data = np.asarray(data, np.float32)
    Bn = data.shape[0]
    cores = []
    for c in range(2 * Bn):
        b, s = c // 2, c % 2
        gy0 = s * 128
        p1 = np.pad(data[b], ((0, 0), (1, 1), (1, 1)))
        i2c = np.stack([p1[:, gy0 + dy:gy0 + dy + 128, dx:dx + 256]
                        for dy in range(3) for dx in range(3)], axis=0).reshape(27, 128, 256)
        p2 = np.pad(data[b], ((0, 0), (2, 2), (2, 2)))
        kc = np.ascontiguousarray(p2[:, gy0:gy0 + 132, :].transpose(1, 0, 2))
        m_top = 1.0 if s == 1 else 0.0
        m_bot = 1.0 if s == 0 else 0.0
        masks = np.zeros((128, 6), np.float32)
        masks[:, 0] = m_top
        masks[:, 1] = m_bot
        masks[:, 2] = 0.25 * (1 - m_top)
        masks[:, 3] = 0.25 * (1 - m_bot)
        masks[:, 4] = 1 - m_top
        masks[:, 5] = 1 - m_bot
        cores.append({"x_i2c": np.ascontiguousarray(i2c), "data_kc": kc, "masks": masks})
    return cores


def build(n_cores=N_CORES, with_cc=True, taps=(), stages=None):
    import concourse.bass as bass  # noqa: F401
    import concourse.tile as tile
    from concourse import bacc, mybir

    f32 = mybir.dt.float32
    f32r = mybir.dt.float32r
    ADD = mybir.AluOpType.add
    MULT = mybir.AluOpType.mult
    AF = mybir.ActivationFunctionType
    pairs = [[2 * i, 2 * i + 1] for i in range(n_cores // 2)]

    nc = bacc.Bacc("TRN2", target_bir_lowering=False, debug=False,
                   num_devices=n_cores)

    x_i2c = nc.dram_tensor("x_i2c", [27, 128, 256], f32, kind="ExternalInput")
    data_kc = nc.dram_tensor("data_kc", [132, 3, 260], f32, kind="ExternalInput")
    masks = nc.dram_tensor("masks", [128, 6], f32, kind="ExternalInput")
    w_in, b_in = {}, {}
    for name, Cin, Cout, n, W, relu, srcs, ksize, nb in CFG:
        nci_tot = sum(x[1] for x in srcs)
        ci = Cin // nci_tot
        nco, co = nsplit(Cout)
        nsh = 1 if ksize == 1 else 9
        w_in[name] = nc.dram_tensor(f"w_{name}", [nco, ci, nci_tot, nsh, co], f32,
                                    kind="ExternalInput")
        b_in[name] = nc.dram_tensor(f"b_{name}", [co, nco], f32, kind="ExternalInput")
    out_t = nc.dram_tensor("out", [3, 128, 256], f32, kind="ExternalOutput")
    tap_t = {k: nc.dram_tensor(f"tap_{k}", [nsplit(HOMES[k][0])[0], nsplit(HOMES[k][0])[1],
                                            HOMES[k][1] + 2, HOMES[k][2]], f32,
                               kind="ExternalOutput")
             for k in taps}

    with tile.TileContext(nc) as tc:
        from contextlib import ExitStack
        with ExitStack() as ctx:
            gp = ctx.enter_context(tc.tile_pool(name="glob", bufs=1))
            pspool = ctx.enter_context(tc.tile_pool(name="psum", bufs=8, space="PSUM"))
            agp = ctx.enter_context(tc.tile_pool(name="agst", bufs=2))
            drp = ctx.enter_context(tc.tile_pool(name="drbounce", bufs=2, space="DRAM"))
            drh = ctx.enter_context(tc.tile_pool(name="drhomes", bufs=1, space="DRAM"))

            m_sb = gp.tile([128, 6], f32, tag="m_sb")
            nc.sync.dma_start(m_sb[:], masks[:])

            homes = {"x_i2c": x_i2c}
            for key, (Ctot, n, W) in HOMES.items():
                nci, ci = nsplit(Ctot)
                homes[key] = drh.tile([nci, ci, n + 2, W], f32, tag=key, name=key)
            u8t = drh.tile([128, 75, 256], f32, tag="h_u8t", name="h_u8t")

            def emit_conv(name):
                _, Cin, Cout, n, W, relu, srcs, ksize, nb = CFGD[name]
                ex = CONV_EXTRAS.get(name, {})
                full_rows = ex.get("full_rows", False)
                pool_dst = homes[ex["pool_dst"]] if "pool_dst" in ex else None
                skip_home = ex.get("skip_home", False)
                nci_tot = sum(x[1] for x in srcs)
                ci = Cin // nci_tot
                nco, co = nsplit(Cout)
                nsh = 1 if ksize == 1 else 9
                rch = min(RCHUNK[W] if nsh == 1 else RCH3[W], n)
                br = n // nb
                raw = (srcs[0][0] == "x_i2c")
                dst = homes["h_oc8"] if name == "outc" else (
                    None if skip_home else homes[f"h_{name}"])
                orows = br + 2 if full_rows else br
                with tc.tile_pool(name=f"p_{name}", bufs=2) as lp, \
                     tc.tile_pool(name=f"pw_{name}", bufs=2) as wp:
                    b_t = lp.tile([co, nco], f32, tag="b")
                    nc.sync.dma_start(b_t[:], b_in[name][:])
                    for ib in range(nb):
                        rb = ib * br
                        wp2 = W + 2
                        if raw:
                            x_t = lp.tile([ci, br, W], f32r, tag="x")
                            nc.sync.dma_start(x_t[:], x_i2c[:, rb:rb + br, :].bitcast(f32r))
                        elif nsh == 1:
                            x_t = lp.tile([ci, nci_tot, br + 2, W], f32r, tag="x")
                            gg = 0
                            for hname, hnci in srcs:
                                hm = homes[hname]
                                nc.sync.dma_start(
                                    x_t[:, gg:gg + hnci],
                                    hm[:, :, rb:rb + br + 2, :]
                                    .rearrange("g c r w -> c g r w").bitcast(f32r))
                                gg += hnci
                        else:
                            # flat im2col rows of width W+2; the 2 extra cols are
                            # zeroed so every 3x3 shift is one contiguous matmul
                            # with exact zero-pad column semantics. Row 0 / row
                            # br+3 are junk pad rows (only their zero cols and a
                            # single wrap element reach the psum garbage cols).
                            x_t = lp.tile([ci, nci_tot, br + 4, wp2], f32r, tag="x")
                            nc.vector.memset(x_t[:, :, :, W:W + 2].bitcast(f32), 0.0)
                            gg = 0
                            for hname, hnci in srcs:
                                hm = homes[hname]
                                for g in range(hnci):
                                    nc.sync.dma_start(
                                        x_t[:, gg, 1:br + 3, 0:W],
                                        hm[g, :, rb:rb + br + 2, :].bitcast(f32r))
                                    gg += 1
                        chunks = [(rc, min(rch, orows - rc))
                                  for rc in range(0, orows, rch)]
                        # multi-group convs loop groups outer (one small weight
                        # tile live at a time); psum tiles persist across groups
                        gg_outer = (nci_tot >= 2)
                        assert not gg_outer or len(chunks) <= 8
                        for o in range(nco):
                            o_t = lp.tile([co, orows, W], f32, tag="o")
                            if gg_outer:
                                pss = []
                                for k, (rc, rr) in enumerate(chunks):
                                    pss.append(pspool.tile([128, rch, wp2], f32,
                                                           tag="ps", name=f"ps{k}"))
                                for gg in range(nci_tot):
                                    w_t = wp.tile([ci, nsh, co], f32r, tag="w")
                                    nc.sync.dma_start(
                                        w_t[:], w_in[name][o, :, gg].bitcast(f32r))
                                    x_flat = x_t[:, gg].rearrange("c r w -> c (r w)")
                                    for k, (rc, rr) in enumerate(chunks):
                                        for s in range(9):
                                            dy, dx = s // 3, s % 3
                                            a = (1 + rc + dy) * wp2 + dx - 1
                                            rhs = x_flat[:, a:a + rr * wp2]
                                            nc.tensor.matmul(
                                                pss[k][:co, :rr], w_t[:, s, :], rhs,
                                                start=(gg == 0 and s == 0),
                                                stop=(gg == nci_tot - 1 and s == 8))
                                for k, (rc, rr) in enumerate(chunks):
                                    nc.scalar.activation(o_t[:, rc:rc + rr, :],
                                                         pss[k][:co, :rr, 0:W],
                                                         AF.Relu, bias=b_t[:, o:o + 1])
                            else:
                                w_t = wp.tile([ci, nsh, co], f32r, tag="w")
                                nc.sync.dma_start(w_t[:],
                                                  w_in[name][o, :, 0].bitcast(f32r))
                                for rc, rr in chunks:
                                    if nsh == 1:
                                        ps = pspool.tile([128, rch, W], f32, tag="ps")
                                        pvalid = ps[:co, :rr]
                                        if raw:
                                            rhs = x_t[:, rc:rc + rr, :]
                                        elif full_rows:
                                            rhs = x_t[:, 0, rc:rc + rr, :]
                                        else:
                                            rhs = x_t[:, 0, rc + 1:rc + 1 + rr, :]
                                        nc.tensor.matmul(pvalid, w_t[:, 0, :], rhs,
                                                         start=True, stop=True)
                                    else:
                                        ps = pspool.tile([128, rch, wp2], f32, tag="ps")
                                        pvalid = ps[:co, :rr, 0:W]
                                        x_flat = x_t[:, 0].rearrange("c r w -> c (r w)")
                                        for s in range(9):
                                            dy, dx = s // 3, s % 3
                                            a = (1 + rc + dy) * wp2 + dx - 1
                                            rhs = x_flat[:, a:a + rr * wp2]
                                            nc.tensor.matmul(ps[:co, :rr], w_t[:, s, :],
                                                             rhs, start=(s == 0),
                                                             stop=(s == 8))
                                    if relu:
                                        nc.scalar.activation(o_t[:, rc:rc + rr, :], pvalid,
                                                             AF.Relu, bias=b_t[:, o:o + 1])
                                    else:
                                        nc.vector.tensor_scalar_add(o_t[:, rc:rc + rr, :],
                                                                    pvalid, b_t[:, o:o + 1])
                            if full_rows:
                                nc.sync.dma_start(dst[o, :, rb:rb + br + 2, :], o_t[:])
                            elif dst is not None:
                                nc.sync.dma_start(dst[o, :, 1 + rb:1 + rb + br, :], o_t[:])
                            if pool_dst is not None:
                                pt1 = lp.tile([co, br // 2, W], f32, tag="pt1")
                                nc.vector.tensor_tensor(pt1[:], o_t[:, 0:br:2, :],
                                                        o_t[:, 1:br:2, :], ADD)
                                pt2 = lp.tile([co, br // 2, W // 2], f32, tag="pt2")
                                nc.vector.tensor_tensor(pt2[:], pt1[:, :, 0:W:2],
                                                        pt1[:, :, 1:W:2], ADD)
                                po = lp.tile([co, br // 2, W // 2], f32, tag="po")
                                nc.scalar.activation(po[:], pt2[:], AF.Copy, scale=0.25)
                                nc.sync.dma_start(
                                    pool_dst[o, :, 1 + rb // 2:1 + rb // 2 + br // 2, :],
                                    po[:])

            def emit_ag(key, Ctot, n, W):
                nci, ci = nsplit(Ctot)
                hm = homes[key]
                C = nci * ci
                bin_ = drp.tile([2, C, W], f32, tag="bin", name=f"bin_{key}")
                bout = drp.tile([2, 2, C, W], f32, tag="bout", name=f"bout_{key}")
                nc.sync.dma_start(bin_[0], hm[:, :, 1, :].rearrange("g c w -> (g c) w"))
                nc.sync.dma_start(bin_[1], hm[:, :, n, :].rearrange("g c w -> (g c) w"))
                if with_cc:
                    nc.gpsimd.collective_compute(
                        "AllGather", mybir.AluOpType.bypass, replica_groups=pairs,
                        ins=[bin_.opt()], outs=[bout.opt()])
                else:
                    nc.sync.dma_start(bout[0], bin_[:])
                    nc.sync.dma_start(bout[1], bin_[:])
                st = agp.tile([ci, 2, nci, W], f32, tag="st")
                nc.sync.dma_start(st[:, 0], bout[0, 1].rearrange("(g c) w -> c g w", c=ci))
                nc.sync.dma_start(st[:, 1], bout[1, 0].rearrange("(g c) w -> c g w", c=ci))
                nc.vector.tensor_scalar_mul(st[:, 0], st[:, 0], m_sb[:ci, 0:1])
                nc.vector.tensor_scalar_mul(st[:, 1], st[:, 1], m_sb[:ci, 1:2])
                nc.sync.dma_start(hm[:, :, 0, :].rearrange("g c w -> c g w"), st[:, 0])
                nc.sync.dma_start(hm[:, :, n + 1, :].rearrange("g c w -> c g w"), st[:, 1])

            def emit_pool(srck, dstk, Ctot, n, W, nb):
                nci, ci = nsplit(Ctot)
                hW = W // 2
                br = n // nb
                src, dst = homes[srck], homes[dstk]
                with tc.tile_pool(name=f"pl_{dstk}", bufs=2) as lp:
                    for g in range(nci):
                        for ib in range(nb):
                            rb = ib * br
                            x = lp.tile([ci, br, W], f32, tag="x")
                            nc.sync.dma_start(x[:], src[g, :, 1 + rb:1 + rb + br, :])
                            t1 = lp.tile([ci, br // 2, W], f32, tag="t1")
                            nc.vector.tensor_tensor(t1[:], x[:, 0:br:2, :], x[:, 1:br:2, :], ADD)
                            t2 = lp.tile([ci, br // 2, hW], f32, tag="t2")
                            nc.vector.tensor_tensor(t2[:], t1[:, :, 0:W:2],
                                                    t1[:, :, 1:W:2], ADD)
                            o = lp.tile([ci, br // 2, hW], f32, tag="o")
                            nc.scalar.activation(o[:], t2[:], AF.Copy, scale=0.25)
                            nc.sync.dma_start(
                                dst[g, :, 1 + rb // 2:1 + rb // 2 + br // 2, :], o[:])

            def emit_up2(srck, dstk, Ctot, n, W, tb, transposed=False):
                nci, ci = nsplit(Ctot)
                Wp2 = W
                n2 = 2 * n + 2
                src = homes[srck]
                dst = u8t if transposed else homes[dstk]
                with tc.tile_pool(name=f"up_{dstk}", bufs=1) as lp:
                    for g in range(nci):
                        for t0 in range(0, n2, tb):
                            t1_ = min(t0 + tb, n2)
                            tbn = t1_ - t0
                            k0 = t0 // 2
                            kn = t1_ // 2 - k0 + 1
                            half = tbn // 2
                            x = lp.tile([ci, tb // 2 + 1, Wp2], f32, tag="x")
                            nc.sync.dma_start(x[:, :kn], src[g, :, k0:k0 + kn, :])
                            Bq = lp.tile([ci, tb // 2 + 1, Wp2], f32, tag="B")
                            nc.vector.tensor_scalar_mul(Bq[:, :kn], x[:, :kn], 0.25)
                            nc.scalar.activation(x[:, :kn], x[:, :kn], AF.Copy, scale=0.75)
                            T = lp.tile([ci, tb, Wp2], f32, tag="T")
                            nc.vector.tensor_tensor(T[:, 0:tbn:2, :], x[:, 0:half, :],
                                                    Bq[:, 1:half + 1, :], ADD)
                            nc.vector.tensor_tensor(T[:, 1:tbn:2, :], x[:, 1:half + 1, :],
                                                    Bq[:, 0:half, :], ADD)
                            if t0 == 0 and not transposed:
                                nc.vector.tensor_scalar_mul(T[:, 0:1, :], T[:, 0:1, :],
                                                            m_sb[:ci, 0:1])
                            if t0 <= 1 < t1_:
                                c1t = lp.tile([ci, 1, Wp2], f32, tag="c1")
                                nc.vector.tensor_scalar_mul(c1t[:], Bq[:, 1 - k0:2 - k0, :],
                                                            m_sb[:ci, 4:5])
                                nc.vector.tensor_tensor(T[:, 1 - t0:2 - t0, :],
                                                        T[:, 1 - t0:2 - t0, :], c1t[:], ADD)
                            if t0 <= 2 * n < t1_:
                                c2t = lp.tile([ci, 1, Wp2], f32, tag="c2")
                                nc.vector.tensor_scalar_mul(c2t[:], Bq[:, n - k0:n - k0 + 1, :],
                                                            m_sb[:ci, 5:6])
                                lo = 2 * n - t0
                                nc.vector.tensor_tensor(T[:, lo:lo + 1, :],
                                                        T[:, lo:lo + 1, :], c2t[:], ADD)
                            if t1_ == n2 and not transposed:
                                nc.vector.tensor_scalar_mul(T[:, tbn - 1:tbn, :],
                                                            T[:, tbn - 1:tbn, :], m_sb[:ci, 1:2])
                            B2 = lp.tile([ci, tb, Wp2], f32, tag="B2")
                            nc.vector.tensor_scalar_mul(B2[:, :tbn], T[:, :tbn], 0.25)
                            nc.scalar.activation(T[:, :tbn], T[:, :tbn], AF.Copy, scale=0.75)
                            Y = lp.tile([ci, tb, 2 * W], f32, tag="Y")
                            nc.vector.tensor_tensor(Y[:, :tbn, 2:2 * W:2], T[:, :tbn, 1:W],
                                                    B2[:, :tbn, 0:W - 1], ADD)
                            nc.vector.tensor_tensor(Y[:, :tbn, 0:1], T[:, :tbn, 0:1],
                                                    B2[:, :tbn, 0:1], ADD)
                            nc.vector.tensor_tensor(Y[:, :tbn, 1:2 * W - 1:2], T[:, :tbn, 0:W - 1],
                                                    B2[:, :tbn, 1:W], ADD)
                            nc.vector.tensor_tensor(Y[:, :tbn, 2 * W - 1:2 * W],
                                                    T[:, :tbn, W - 1:W],
                                                    B2[:, :tbn, W - 1:W], ADD)
                            if transposed:
                                loc0 = 1 if t0 == 0 else 0
                                loc1 = tbn - 1 if t1_ == n2 else tbn
                                y0 = t0 + loc0 - 1
                                nc.sync.dma_start(
                                    dst[y0:y0 + (loc1 - loc0)]
                                    .rearrange("y c x -> c y x"),
                                    Y[:, loc0:loc1])
                            else:
                                nc.sync.dma_start(dst[g, :, t0:t1_, :], Y[:, :tbn])

            def emit_kc():
                with tc.tile_pool(name="kc", bufs=1) as lp:
                    ct = lp.tile([128, 75, 256], f32, tag="ct")
                    nc.sync.dma_start(ct[:], u8t[:])
                    acc = lp.tile([128, 3, 256], f32, tag="acc")
                    tmp = lp.tile([128, 3, 256], f32, tag="tmp")
                    first = True
                    for i in range(5):
                        d_i = lp.tile([128, 3, 260], f32, tag=f"d{i}")
                        nc.sync.dma_start(d_i[:], data_kc[i:i + 128])
                        for j in range(5):
                            kk = i * 5 + j
                            if first:
                                nc.vector.tensor_tensor(acc[:], ct[:, kk * 3:(kk + 1) * 3, :],
                                                        d_i[:, :, j:j + 256], MULT)
                                first = False
                            else:
                                nc.vector.tensor_tensor(tmp[:], ct[:, kk * 3:(kk + 1) * 3, :],
                                                        d_i[:, :, j:j + 256], MULT)
                                nc.vector.tensor_tensor(acc[:], acc[:], tmp[:], ADD)
                    nc.sync.dma_start(out_t.rearrange("c y x -> y c x"), acc[:])

            for st in (STAGES if stages is None else STAGES[:stages]):
                if st[0] == "conv":
                    emit_conv(st[1])
                elif st[0] == "ag":
                    emit_ag(*st[1:])
                elif st[0] == "pool":
                    emit_pool(*st[1:])
                elif st[0] == "up":
                    emit_up2(*st[1:])
                elif st[0] == "upt":
                    emit_up2(*st[1:], transposed=True)
                elif st[0] == "kc":
                    emit_kc()

            for k in taps:
                hm = homes[k]
                nci, ci = nsplit(HOMES[k][0])
                for g in range(nci):
                    nc.sync.dma_start(tap_t[k][g], hm[g])

    nc.finalize()
    return nc


class SpmdRunner:
    def __init__(self, nc, n_cores, sharded_names):
        import jax
        import numpy as _np
        from jax.sharding import Mesh, PartitionSpec
        from jax.experimental.shard_map import shard_map
        import concourse.mybir as mybir
        from concourse.bass2jax import (_bass_exec_p, partition_id_tensor,
                                        install_neuronx_cc_hook)
        install_neuronx_cc_hook()
        self.jax = jax
        self.n_cores = n_cores
        self.sharded = set(sharded_names)
        partition_name = nc.partition_id_tensor.name if nc.partition_id_tensor else None
        in_names, out_names, out_avals, zero_outs = [], [], [], []
        for alloc in nc.m.functions[0].allocations:
            if not isinstance(alloc, mybir.MemoryLocationSet):
                continue
            name = alloc.memorylocations[0].name
            if alloc.kind == "ExternalInput":
                if name != partition_name:
                    in_names.append(name)
            elif alloc.kind == "ExternalOutput":
                out_names.append(name)
                shape = tuple(alloc.tensor_shape)
                dtype = mybir.dt.np(alloc.dtype)
                out_avals.append(jax.core.ShapedArray(shape, dtype))
                zero_outs.append(_np.zeros(shape, dtype))
        self.in_names, self.out_names = in_names, out_names
        self.out_avals, self.zero_outs = out_avals, zero_outs
        n_params, n_outs = len(in_names), len(out_avals)
        all_in = list(in_names) + list(out_names)
        if partition_name is not None:
            all_in.append(partition_name)
        all_in = tuple(all_in)

        def _body(*args):
            operands = list(args)
            if partition_name is not None:
                operands.append(partition_id_tensor())
            outs = _bass_exec_p.bind(
                *operands, out_avals=tuple(out_avals), in_names=all_in,
                out_names=tuple(out_names), lowering_input_output_aliases=(),
                sim_require_finite=False, sim_require_nnan=False, nc=nc)
            return tuple(outs)

        devices = jax.devices()[:n_cores]
        assert len(devices) == n_cores
        mesh = Mesh(_np.asarray(devices), ("core",))
        in_specs = tuple(
            PartitionSpec("core") if nm in self.sharded else PartitionSpec()
            for nm in in_names) + (PartitionSpec("core"),) * n_outs
        out_specs = (PartitionSpec("core"),) * n_outs
        self._fn = jax.jit(
            shard_map(_body, mesh=mesh, in_specs=in_specs, out_specs=out_specs,
                      check_rep=False), keep_unused=True)

    def prep_args(self, in_maps):
        import numpy as _np
        n = self.n_cores
        args = []
        for nm in self.in_names:
            if nm in self.sharded:
                args.append(_np.concatenate(
                    [_np.asarray(in_maps[c][nm]) for c in range(n)], axis=0))
            else:
                args.append(_np.asarray(in_maps[0][nm]))
        args += [_np.zeros((n * z.shape[0], *z.shape[1:]), z.dtype)
                 for z in self.zero_outs]
        return args

    def run(self, in_maps):
        import numpy as _np
        n = self.n_cores
        outs = self._fn(*self.prep_args(in_maps))
        self.jax.block_until_ready(outs)
        return [
            {k: _np.asarray(outs[i]).reshape(n, *self.out_avals[i].shape)[c]
             for i, k in enumerate(self.out_names)}
            for c in range(n)
        ]

    def device_args(self, in_maps):
        import numpy as _np
        from jax.sharding import Mesh, PartitionSpec, NamedSharding
        jax = self.jax
        mesh = Mesh(_np.asarray(jax.devices()[:self.n_cores]), ("core",))
        args = self.prep_args(in_maps)
        specs = [PartitionSpec("core") if nm in self.sharded else PartitionSpec()
                 for nm in self.in_names]
        specs += [PartitionSpec("core")] * len(self.zero_outs)
        return [jax.device_put(a, NamedSharding(mesh, s))
                for a, s in zip(args, specs)]

    def timeit(self, in_maps, reps=10):
        import time
        args = self.device_args(in_maps)
        outs = self._fn(*args)
        self.jax.block_until_ready(outs)
        ts = []
        for _ in range(reps):
            t0 = time.perf_counter()
            outs = self._fn(*args)
            self.jax.block_until_ready(outs)
            ts.append(time.perf_counter() - t0)
        return ts


_STATE = {}


def _get_runner(taps=()):
    key = ("runner", tuple(taps))
    if key not in _STATE:
        nc = build(N_CORES, with_cc=True, taps=taps)
        _STATE[key] = SpmdRunner(nc, N_CORES,
                                 sharded_names=["x_i2c", "data_kc", "masks"])
    return _STATE[key]


def kernel(data, params):
    data = np.asarray(data, np.float32)
    Bn = data.shape[0]
    assert Bn * 2 == N_CORES
    wpk = pack_weights(params)
    cores = per_core_inputs(data)
    in_maps = []
    for c in range(N_CORES):
        m = dict(cores[c])
        m.update(wpk)
        in_maps.append(m)
    runner = _get_runner(taps=_STATE.get("taps", ()))
    res = runner.run(in_maps)
    full = np.zeros((Bn, 3, 256, 256), np.float32)
    for c in range(N_CORES):
        b, s = c // 2, c % 2
        full[b, :, s * 128:(s + 1) * 128, :] = res[c]["out"]
    _STATE["last_res"] = res
    return full
